# revision 2
# baseline (speedup 1.0000x reference)
"""Trainium2 Bass kernel for nn_CachePredictor (moe_routing).

Computation (see reference):
    x = relu(feature @ W_up.T + b_up)                      [B, 512]
    t_out = sigmoid(einsum('bf,bgf', x, W_table[tids]) + b_table[tids]) * tmask
    i_out = sigmoid(einsum('bf,bgf', x, W_index[iids]) + b_index[iids]) * imask
    out = stack([t_out, i_out])                            [2, B, 256]

Strategy: expert sharding. Per-sample gather of expert weights would move
~4 GB of HBM traffic; grouping samples by expert reads each expert matrix
exactly once. Each of the 8 cores owns 8 table experts and 16 index
experts and processes only the samples routed to its experts. The host
computes routing metadata (sample->expert grouping, capacity padding) and
arranges per-core inputs; all FLOPs run on device.

v2 changes vs the 41 us baseline (toward the ~13 us DMA / ~14 us PE
rooflines):
- Expert weights stored in HBM as fp8 e3m4 (scaled x32, clipped to +-15.5)
  and fed STRAIGHT to the PE as the moving operand while x stays bf16
  (mixed-dtype matmul verified on HW). Halves the dominant HBM stream
  (6.3 -> 3.15 MB/core). The x32 is compensated exactly by folding /32
  into W_up/b_up (pure exponent shift in bf16).
- No warmup matmuls, no K=1 bias matmuls (~9 us of PE instructions in the
  baseline). Expert biases are partition_broadcast once on the idle GPSIMD
  into a [128, 6144] bf16 tile; per pair the bias-add runs on the DVE
  (PSUM + bias -> SBUF f32) and the sigmoid on ACT (-> bf16).
- Outputs written bf16 (host upcasts): halves output traffic.
- Weight chunks of 8 experts (1 MB fp8) stream on the sync HWDGE ring
  only, so ACT compute never delays a weight-DMA dispatch; bulk outputs
  ride SWDGE, the final chunk's output uses the scalar ring for a short
  tail.
- A dummy 1-element sigmoid early on ACT pulls the ~1.3 us activation
  table load into the startup window.

Per-PAIR processing (pair = 2 consecutive experts sharing one padded
column segment = union of both experts' samples): every sample is
multiplied against BOTH experts' weights in one N=512 moving pass and the
host keeps the valid half. Redundant FLOPs are cheap; PE instruction
count is what matters.

Masked-off samples are never routed (reference zeroes them); the host
scatters computed rows back and leaves the rest zero.
"""

import ml_dtypes
import numpy as np

_N_CORES = 8
_F = 256        # feature dim
_HID = 512      # up-projection width
_G = 256        # buckets
_N_TABLES = 64
_N_INDEXES = 128
_TPC = _N_TABLES // _N_CORES    # table experts per core (8)
_IPC = _N_INDEXES // _N_CORES   # index experts per core (16)
_CPE = 8                        # experts per weight chunk (1 MiB fp8)
_WSCALE = 32.0                  # fp8 weight scale (folded into W_up/b_up)

_nc_cache = {}

# Set by a test harness to capture HW profiles; harmless when unused.
TRACE = False
LAST_RESULTS = None


def _build(Cpt, Cpi):
    """Build + compile the SPMD program for per-PAIR capacities (Cpt, Cpi)."""
    from concourse import bacc
    import concourse.tile as tile
    import concourse.mybir as mybir

    F32 = mybir.dt.float32
    BF16 = mybir.dt.bfloat16
    F8E3 = mybir.dt.float8e3
    AF = mybir.ActivationFunctionType

    TP = _TPC // 2   # table pairs per core (4)
    IP = _IPC // 2   # index pairs per core (8)
    NTcols = TP * Cpt
    NIcols = IP * Cpi
    TCH = _TPC // _CPE   # table weight chunks (1)
    ICH = _IPC // _CPE   # index weight chunks (2)
    PRS = _CPE // 2      # pairs per chunk (4)
    NB = (_TPC + _IPC) * _G   # bias columns (6144)

    nc = bacc.Bacc(
        "TRN2",
        target_bir_lowering=False,
        debug=False,
        enable_asserts=False,
        num_devices=_N_CORES,
    )
    fa = nc.dram_tensor("fa", [_F, NTcols + NIcols], BF16, kind="ExternalInput").ap()
    # host-packed, partition-major: [chunk, p, e_local*1024 + c*256 + g]
    wt = nc.dram_tensor("wt", [TCH, 128, _CPE * 4 * _G], F8E3, kind="ExternalInput").ap()
    wi = nc.dram_tensor("wi", [ICH, 128, _CPE * 4 * _G], F8E3, kind="ExternalInput").ap()
    # biases: table pairs then index pairs, [1, 6144] bf16
    bb = nc.dram_tensor("bb", [1, NB], BF16, kind="ExternalInput").ap()
    wu = nc.dram_tensor("wu", [_F, _HID], BF16, kind="ExternalInput").ap()  # W_up.T/32
    buc = nc.dram_tensor("buc", [128, 4], F32, kind="ExternalInput").ap()  # b_up/32 col-major
    # outputs: per pair, both experts' logits for every sample in the segment
    ot = nc.dram_tensor("ot", [NTcols, 2 * _G], BF16, kind="ExternalOutput").ap()
    oi = nc.dram_tensor("oi", [NIcols, 2 * _G], BF16, kind="ExternalOutput").ap()

    otv = ot.rearrange("(j s) g -> s j g", s=Cpt)
    oiv = oi.rearrange("(j s) g -> s j g", s=Cpi)

    with tile.TileContext(nc) as tc:
        with (
            tc.tile_pool(name="persist", bufs=1) as persist,
            tc.tile_pool(name="wpool", bufs=3) as wpool,
            tc.tile_pool(name="t1pool", bufs=4) as t1pool,
            tc.tile_pool(name="opool", bufs=3) as opool,
            tc.tile_pool(name="ps1pool", bufs=4, space="PSUM") as ps1pool,
            tc.tile_pool(name="ps2pool", bufs=4, space="PSUM") as ps2pool,
        ):
            # feature halves first on each ring so stage 1 can start early
            NA = NTcols + NIcols
            f_sb = []
            for c in range(2):
                f_c = persist.tile([128, NA], BF16, name=f"f_a{c}", tag=f"f_a{c}")
                feng = nc.sync if c == 0 else nc.scalar
                feng.dma_start(out=f_c, in_=fa[c * 128 : (c + 1) * 128, :])
                f_sb.append(f_c)
            wu_sb = persist.tile([128, 2, _HID], BF16, name="wu_sb", tag="wu_sb")
            nc.sync.dma_start(out=wu_sb, in_=wu.rearrange("(c p) m -> p c m", p=128))
            buc_sb = persist.tile([128, 4], F32, name="buc_sb", tag="buc_sb")
            nc.scalar.dma_start(out=buc_sb, in_=buc)
            bb_sb = persist.tile([1, NB], BF16, name="bb_sb", tag="bb_sb")
            nc.gpsimd.dma_start(out=bb_sb, in_=bb)
            # broadcast biases to all partitions once, on the idle GPSIMD
            bbc = persist.tile([128, NB], BF16, name="bbc", tag="bbc")
            nc.gpsimd.partition_broadcast(bbc, bb_sb)

            # pull the ACT sigmoid table load into the startup window
            dummy = persist.tile([1, 16], F32, name="dummy", tag="dummy")
            nc.vector.memset(dummy, 0.0)
            nc.scalar.activation(out=dummy, in_=dummy, func=AF.Sigmoid)

            # stage 1: xT[512, cols] = relu(W_upT.T @ featT + b_up), bf16,
            # paced by DVE relu+bias (one fused tensor_scalar per m-chunk)
            x_sb = {}
            off = {"t": 0, "i": NTcols}
            for role, NC in (("t", NTcols), ("i", NIcols)):
                x_sb[role] = [
                    persist.tile(
                        [128, NC], BF16, name=f"x_{role}{m}", tag=f"x_{role}{m}"
                    )
                    for m in range(4)
                ]
            # interleave m-chunk pairs so consecutive matmuls hit different
            # PSUM banks (same-bank accumulation passes serialize the PE)
            for role, NC in (("t", NTcols), ("i", NIcols)):
                for n0 in range(0, NC, 512):
                    nw = min(512, NC - n0)
                    for m0 in (0, 2):
                        ps1s = {
                            m: ps1pool.tile([128, 512], F32, name="ps1", tag="ps1")
                            for m in (m0, m0 + 1)
                        }
                        for c in range(2):
                            for m in (m0, m0 + 1):
                                nc.tensor.matmul(
                                    ps1s[m][:, :nw],
                                    lhsT=wu_sb[:, c, m * 128 : (m + 1) * 128],
                                    rhs=f_sb[c][:, off[role] + n0 : off[role] + n0 + nw],
                                    start=(c == 0),
                                    stop=(c == 1),
                                )
                        for m in (m0, m0 + 1):
                            nc.vector.tensor_scalar(
                                out=x_sb[role][m][:, n0 : n0 + nw],
                                in0=ps1s[m][:, :nw],
                                scalar1=buc_sb[:, m : m + 1],
                                scalar2=0.0,
                                op0=mybir.AluOpType.add,
                                op1=mybir.AluOpType.max,
                            )

            # stage 2: weight chunks of 8 experts = 4 pairs. One pair => one
            # column segment, one PSUM bank, 4 K-chunk matmuls with rhs
            # spanning both experts (N=512, fp8), DVE bias add, ACT sigmoid.
            for role, wdram, ov, nch, C, boff in (
                ("t", wt, otv, TCH, Cpt, 0),
                ("i", wi, oiv, ICH, Cpi, _TPC * _G),
            ):
                xs = x_sb[role]
                for ch in range(nch):
                    w_sb = wpool.tile(
                        [128, _CPE, 4, _G], F8E3, name=f"w_sb_{role}", tag="w_sb"
                    )
                    wv = wdram[ch].rearrange("p (e c g) -> p e c g", e=_CPE, c=4)
                    # whole chunk on the sync ring: ACT compute never delays
                    # weight streaming
                    nc.sync.dma_start(out=w_sb, in_=wv)
                    last_chunk = role == "i" and ch == nch - 1
                    for s0 in range(0, C, 128):
                        sw = min(128, C - s0)
                        o_sb = opool.tile(
                            [128, PRS, 2 * _G], BF16, name="o_sb", tag="o_sb"
                        )
                        for pr in range(PRS):
                            j = ch * PRS + pr
                            ps2 = ps2pool.tile([128, 512], F32, name="ps2", tag="ps2")
                            for c in range(4):
                                nc.tensor.matmul(
                                    ps2[:sw, :],
                                    lhsT=xs[c][:, j * C + s0 : j * C + s0 + sw],
                                    rhs=w_sb[:, 2 * pr : 2 * pr + 2, c, :],
                                    start=(c == 0),
                                    stop=(c == 3),
                                )
                            t1 = t1pool.tile([128, 512], F32, name="t1", tag="t1")
                            nc.vector.tensor_tensor(
                                out=t1[:sw, :],
                                in0=ps2[:sw, :],
                                in1=bbc[:sw, boff + j * 512 : boff + (j + 1) * 512],
                                op=mybir.AluOpType.add,
                            )
                            nc.scalar.activation(
                                out=o_sb[:sw, pr, :], in_=t1[:sw, :], func=AF.Sigmoid
                            )
                        # one output DMA per (chunk, s-block): bulk on SWDGE,
                        # final chunk on the scalar ring for a short tail
                        oeng = nc.scalar if last_chunk else nc.gpsimd
                        oeng.dma_start(
                            out=ov[s0 : s0 + sw, ch * PRS : (ch + 1) * PRS, :],
                            in_=o_sb[:sw],
                        )

    nc.compile()
    return nc


def _get_nc(Cpt, Cpi):
    key = (Cpt, Cpi)
    if key not in _nc_cache:
        _nc_cache[key] = _build(Cpt, Cpi)
    return _nc_cache[key]


def _pack_weights(W, nexp):
    """[nexp, G, HID] f32 -> [nexp/_CPE, 128, _CPE*4*G] partition-major fp8
    chunks, scaled by _WSCALE and clipped to the e3m4 range."""
    nch = nexp // _CPE
    A = W.reshape(nch, _CPE, _G, 4, 128)          # [ch, e, g, c, p]
    A = np.ascontiguousarray(A.transpose(0, 4, 1, 3, 2))  # [ch, p, e, c, g]
    A = np.clip(A * _WSCALE, -15.5, 15.5)
    return A.reshape(nch, 128, _CPE * 4 * _G).astype(ml_dtypes.float8_e3m4)


def _route(ids, mask, n_experts):
    """Per-PAIR sample lists: pair j owns experts 2j, 2j+1. Returns
    (pair_samples, pair_parity) lists of arrays."""
    samples, parity = [], []
    for j in range(n_experts // 2):
        s0 = np.flatnonzero((ids == 2 * j) & mask)
        s1 = np.flatnonzero((ids == 2 * j + 1) & mask)
        samples.append(np.concatenate([s0, s1]))
        parity.append(np.concatenate([np.zeros(len(s0), np.int64),
                                      np.ones(len(s1), np.int64)]))
    return samples, parity


def kernel(
    feature,
    table_ids,
    index_ids,
    table_mask,
    index_mask,
    W_up,
    b_up,
    W_table,
    b_table,
    W_index,
    b_index,
):
    global LAST_RESULTS
    from concourse.bass_utils import run_bass_kernel_spmd

    feature = np.ascontiguousarray(np.asarray(feature), dtype=np.float32)
    table_ids = np.asarray(table_ids).astype(np.int64)
    index_ids = np.asarray(index_ids).astype(np.int64)
    table_mask = np.asarray(table_mask).astype(bool)
    index_mask = np.asarray(index_mask).astype(bool)
    W_up = np.asarray(W_up, dtype=np.float32)
    b_up = np.asarray(b_up, dtype=np.float32)
    W_table = np.asarray(W_table, dtype=np.float32)
    b_table = np.asarray(b_table, dtype=np.float32)
    W_index = np.asarray(W_index, dtype=np.float32)
    b_index = np.asarray(b_index, dtype=np.float32)

    B = feature.shape[0]

    smp_t, par_t = _route(table_ids, table_mask, _N_TABLES)
    smp_i, par_i = _route(index_ids, index_mask, _N_INDEXES)
    # Uniform per-pair capacity so all 8 cores run one identical program.
    Cpt = max(8, -(-max(len(s) for s in smp_t) // 8) * 8)
    Cpi = max(8, -(-max(len(s) for s in smp_i) // 8) * 8)

    nc = _get_nc(Cpt, Cpi)

    TP = _TPC // 2
    IP = _IPC // 2
    W_upT = np.ascontiguousarray(W_up.T / _WSCALE).astype(ml_dtypes.bfloat16)
    buc = np.ascontiguousarray((b_up / _WSCALE).reshape(4, 128).T)

    in_maps = []
    for c in range(_N_CORES):
        ts = slice(c * _TPC, (c + 1) * _TPC)
        is_ = slice(c * _IPC, (c + 1) * _IPC)
        fa_c = np.zeros((_F, TP * Cpt + IP * Cpi), ml_dtypes.bfloat16)
        ft_c = fa_c[:, : TP * Cpt]
        for j in range(TP):
            s = smp_t[c * TP + j]
            if len(s):
                ft_c[:, j * Cpt : j * Cpt + len(s)] = feature[s].T
        fi_c = fa_c[:, TP * Cpt :]
        for j in range(IP):
            s = smp_i[c * IP + j]
            if len(s):
                fi_c[:, j * Cpi : j * Cpi + len(s)] = feature[s].T
        bb_c = np.concatenate(
            [b_table[ts].reshape(-1), b_index[is_].reshape(-1)]
        ).reshape(1, -1).astype(ml_dtypes.bfloat16)
        in_maps.append(
            {
                "fa": fa_c,
                "wt": _pack_weights(W_table[ts], _TPC),
                "wi": _pack_weights(W_index[is_], _IPC),
                "bb": bb_c,
                "wu": W_upT,
                "buc": buc,
            }
        )

    res = run_bass_kernel_spmd(
        nc, in_maps, core_ids=list(range(_N_CORES)), trace=TRACE
    )
    LAST_RESULTS = res

    out = np.zeros((2, B, _G), np.float32)
    for c in range(_N_CORES):
        rt = res.results[c]["ot"].astype(np.float32)
        ri = res.results[c]["oi"].astype(np.float32)
        for j in range(TP):
            s = smp_t[c * TP + j]
            if len(s):
                rows = rt[j * Cpt : j * Cpt + len(s)].reshape(len(s), 2, _G)
                out[0, s, :] = rows[np.arange(len(s)), par_t[c * TP + j], :]
        for j in range(IP):
            s = smp_i[c * IP + j]
            if len(s):
                rows = ri[j * Cpi : j * Cpi + len(s)].reshape(len(s), 2, _G)
                out[1, s, :] = rows[np.arange(len(s)), par_i[c * IP + j], :]
    return out


# revision 8
# speedup vs baseline: 1.1131x; 1.1131x over previous
"""Trainium2 Bass kernel for nn_CachePredictor (moe_routing).

Computation (see reference):
    x = relu(feature @ W_up.T + b_up)                      [B, 512]
    t_out = sigmoid(einsum('bf,bgf', x, W_table[tids]) + b_table[tids]) * tmask
    i_out = sigmoid(einsum('bf,bgf', x, W_index[iids]) + b_index[iids]) * imask
    out = stack([t_out, i_out])                            [2, B, 256]

Strategy: expert sharding. Per-sample gather of expert weights would move
~4 GB of HBM traffic; grouping samples by expert reads each expert matrix
exactly once. Each of the 8 cores owns 8 table experts and 16 index
experts and processes only the samples routed to its experts. The host
computes routing metadata (sample->expert grouping, capacity padding) and
arranges per-core inputs; all FLOPs run on device.

v2 changes vs the 41 us baseline (toward the ~13 us DMA / ~14 us PE
rooflines):
- Expert weights stored in HBM as fp8 e3m4 (scaled x32, clipped to +-15.5)
  and fed STRAIGHT to the PE as the moving operand while x stays bf16
  (mixed-dtype matmul verified on HW). Halves the dominant HBM stream
  (6.3 -> 3.15 MB/core). The x32 is compensated exactly by folding /32
  into W_up/b_up (pure exponent shift in bf16).
- No warmup matmuls, no K=1 bias matmuls (~9 us of PE instructions in the
  baseline). Expert biases arrive host-pre-broadcast as a [128, 6144]
  fp8e3 input (0.77 MB, abs err <= ~5e-4 on biases ~1e-2); per pair the
  bias-add runs on the DVE (PSUM + bias -> SBUF f32) and the sigmoid on
  ACT (-> bf16). (A device-side gpsimd partition_broadcast was tried and
  measured 9.2 us + bad scheduling - it serialized all of stage 2.)
- Stage-1 relu+bias is split: m-chunks 0,1 on ACT (native Relu with
  per-partition bias), 2,3 on DVE - stage 1 was DVE-paced otherwise.
  Dummy 1-element Relu/Sigmoid ops pull the ~1.3 us ACT table loads off
  the critical path.
- wu rides the scalar ring while fa0 rides sync, so the first stage-1
  matmul's deps land in parallel right after the ~7 us engine preamble.
- Outputs written bf16 (host upcasts): halves output traffic.
- Weight chunks of 8 experts (1 MB fp8) stream on the sync HWDGE ring
  only, so ACT compute never delays a weight-DMA dispatch; bulk outputs
  ride SWDGE, the final chunk's output uses the scalar ring for a short
  tail.
- A dummy 1-element sigmoid early on ACT pulls the ~1.3 us activation
  table load into the startup window.

Per-PAIR processing (pair = 2 consecutive experts sharing one padded
column segment = union of both experts' samples): every sample is
multiplied against BOTH experts' weights in one N=512 moving pass and the
host keeps the valid half. Redundant FLOPs are cheap; PE instruction
count is what matters.

Masked-off samples are never routed (reference zeroes them); the host
scatters computed rows back and leaves the rest zero.
"""

import ml_dtypes
import numpy as np

_N_CORES = 8
_F = 256        # feature dim
_HID = 512      # up-projection width
_G = 256        # buckets
_N_TABLES = 64
_N_INDEXES = 128
_TPC = _N_TABLES // _N_CORES    # table experts per core (8)
_IPC = _N_INDEXES // _N_CORES   # index experts per core (16)
_CPE = 8                        # experts per weight chunk (1 MiB fp8)
_WSCALE = 32.0                  # fp8 weight scale (folded into W_up/b_up)

_nc_cache = {}

# Set by a test harness to capture HW profiles; harmless when unused.
TRACE = False
LAST_RESULTS = None


def _build(Cpt, Cpi):
    """Build + compile the SPMD program for per-PAIR capacities (Cpt, Cpi)."""
    from concourse import bacc
    import concourse.tile as tile
    import concourse.mybir as mybir

    F32 = mybir.dt.float32
    BF16 = mybir.dt.bfloat16
    F8E3 = mybir.dt.float8e3
    AF = mybir.ActivationFunctionType

    TP = _TPC // 2   # table pairs per core (4)
    IP = _IPC // 2   # index pairs per core (8)
    NTcols = TP * Cpt
    NIcols = IP * Cpi
    TCH = _TPC // _CPE   # table weight chunks (1)
    ICH = _IPC // _CPE   # index weight chunks (2)
    PRS = _CPE // 2      # pairs per chunk (4)
    NB = (_TPC + _IPC) * _G   # bias columns (6144)

    nc = bacc.Bacc(
        "TRN2",
        target_bir_lowering=False,
        debug=False,
        enable_asserts=False,
        num_devices=_N_CORES,
    )
    fa = nc.dram_tensor("fa", [_F, NTcols + NIcols], BF16, kind="ExternalInput").ap()
    # host-packed, partition-major: [chunk, p, e_local*1024 + c*256 + g]
    wt = nc.dram_tensor("wt", [TCH, 128, _CPE * 4 * _G], F8E3, kind="ExternalInput").ap()
    wi = nc.dram_tensor("wi", [ICH, 128, _CPE * 4 * _G], F8E3, kind="ExternalInput").ap()
    # biases, host-pre-broadcast to all partitions: table pairs then index
    # pairs, [128, 6144] fp8e3
    bb = nc.dram_tensor("bb", [128, NB], F8E3, kind="ExternalInput").ap()
    wu = nc.dram_tensor("wu", [_F, _HID], BF16, kind="ExternalInput").ap()  # W_up.T/32
    buc = nc.dram_tensor("buc", [128, 4], F32, kind="ExternalInput").ap()  # b_up/32 col-major
    # outputs: per pair, both experts' logits for every sample in the segment
    ot = nc.dram_tensor("ot", [NTcols, 2 * _G], BF16, kind="ExternalOutput").ap()
    oi = nc.dram_tensor("oi", [NIcols, 2 * _G], BF16, kind="ExternalOutput").ap()

    otv = ot.rearrange("(j s) g -> s j g", s=Cpt)
    oiv = oi.rearrange("(j s) g -> s j g", s=Cpi)

    with tile.TileContext(nc) as tc:
        with (
            tc.tile_pool(name="persist", bufs=1) as persist,
            tc.tile_pool(name="wpool", bufs=3) as wpool,
            tc.tile_pool(name="t1pool", bufs=4) as t1pool,
            tc.tile_pool(name="opool", bufs=3) as opool,
            tc.tile_pool(name="ps1pool", bufs=4, space="PSUM") as ps1pool,
            tc.tile_pool(name="ps2pool", bufs=4, space="PSUM") as ps2pool,
        ):
            # first stage-1 matmul needs wu + fa0: land them in parallel
            # (wu on the scalar ring, fa0 on sync, fa1 after wu on scalar)
            NA = NTcols + NIcols
            wu_sb = persist.tile([128, 2, _HID], BF16, name="wu_sb", tag="wu_sb")
            nc.scalar.dma_start(out=wu_sb, in_=wu.rearrange("(c p) m -> p c m", p=128))
            buc_sb = persist.tile([128, 4], F32, name="buc_sb", tag="buc_sb")
            nc.scalar.dma_start(out=buc_sb, in_=buc)
            f_sb = []
            for c in range(2):
                f_c = persist.tile([128, NA], BF16, name=f"f_a{c}", tag=f"f_a{c}")
                feng = nc.sync if c == 0 else nc.scalar
                feng.dma_start(out=f_c, in_=fa[c * 128 : (c + 1) * 128, :])
                f_sb.append(f_c)
            bbc = persist.tile([128, NB], F8E3, name="bbc", tag="bbc")
            nc.scalar.dma_start(out=bbc, in_=bb)

            # pull the ACT relu table load into the startup window
            dummy = persist.tile([1, 16], F32, name="dummy", tag="dummy")
            nc.vector.memset(dummy, 0.0)
            nc.scalar.activation(out=dummy, in_=dummy, func=AF.Relu)

            # stage 1: xT[512, cols] = relu(W_upT.T @ featT + b_up), bf16,
            # paced by DVE relu+bias (one fused tensor_scalar per m-chunk)
            x_sb = {}
            off = {"t": 0, "i": NTcols}
            for role, NC in (("t", NTcols), ("i", NIcols)):
                x_sb[role] = [
                    persist.tile(
                        [128, NC], BF16, name=f"x_{role}{m}", tag=f"x_{role}{m}"
                    )
                    for m in range(4)
                ]
            # interleave m-chunk pairs so consecutive matmuls hit different
            # PSUM banks (same-bank accumulation passes serialize the PE)
            for role, NC in (("t", NTcols), ("i", NIcols)):
                for n0 in range(0, NC, 512):
                    nw = min(512, NC - n0)
                    for m0 in (0, 2):
                        ps1s = {
                            m: ps1pool.tile([128, 512], F32, name="ps1", tag="ps1")
                            for m in (m0, m0 + 1)
                        }
                        for c in range(2):
                            for m in (m0, m0 + 1):
                                nc.tensor.matmul(
                                    ps1s[m][:, :nw],
                                    lhsT=wu_sb[:, c, m * 128 : (m + 1) * 128],
                                    rhs=f_sb[c][:, off[role] + n0 : off[role] + n0 + nw],
                                    start=(c == 0),
                                    stop=(c == 1),
                                )
                        for m in (m0, m0 + 1):
                            # relu+bias split across ACT (native Relu with
                            # per-partition bias) and DVE so neither engine
                            # paces stage 1 alone
                            if m < 2:
                                nc.scalar.activation(
                                    out=x_sb[role][m][:, n0 : n0 + nw],
                                    in_=ps1s[m][:, :nw],
                                    func=AF.Relu,
                                    bias=buc_sb[:, m : m + 1],
                                )
                            else:
                                nc.vector.tensor_scalar(
                                    out=x_sb[role][m][:, n0 : n0 + nw],
                                    in0=ps1s[m][:, :nw],
                                    scalar1=buc_sb[:, m : m + 1],
                                    scalar2=0.0,
                                    op0=mybir.AluOpType.add,
                                    op1=mybir.AluOpType.max,
                                )

            # preload the sigmoid table while stage-2 matmuls run
            nc.scalar.activation(out=dummy, in_=dummy, func=AF.Sigmoid)

            # stage 2: weight chunks of 8 experts = 4 pairs. One pair => one
            # column segment, one PSUM bank, 4 K-chunk matmuls with rhs
            # spanning both experts (N=512, fp8), DVE bias add, ACT sigmoid.
            for role, wdram, ov, nch, C, boff in (
                ("t", wt, otv, TCH, Cpt, 0),
                ("i", wi, oiv, ICH, Cpi, _TPC * _G),
            ):
                xs = x_sb[role]
                for ch in range(nch):
                    w_sb = wpool.tile(
                        [128, _CPE, 4, _G], F8E3, name=f"w_sb_{role}", tag="w_sb"
                    )
                    wv = wdram[ch].rearrange("p (e c g) -> p e c g", e=_CPE, c=4)
                    # whole chunk on the sync ring: ACT compute never delays
                    # weight streaming
                    nc.sync.dma_start(out=w_sb, in_=wv)
                    last_chunk = role == "i" and ch == nch - 1
                    for s0 in range(0, C, 128):
                        sw = min(128, C - s0)
                        o_sb = opool.tile(
                            [128, PRS, 2 * _G], BF16, name="o_sb", tag="o_sb"
                        )
                        for pr in range(PRS):
                            j = ch * PRS + pr
                            ps2 = ps2pool.tile([128, 512], F32, name="ps2", tag="ps2")
                            for c in range(4):
                                nc.tensor.matmul(
                                    ps2[:sw, :],
                                    lhsT=xs[c][:, j * C + s0 : j * C + s0 + sw],
                                    rhs=w_sb[:, 2 * pr : 2 * pr + 2, c, :],
                                    start=(c == 0),
                                    stop=(c == 3),
                                )
                            t1 = t1pool.tile([128, 512], F32, name="t1", tag="t1")
                            nc.vector.tensor_tensor(
                                out=t1[:sw, :],
                                in0=ps2[:sw, :],
                                in1=bbc[:sw, boff + j * 512 : boff + (j + 1) * 512],
                                op=mybir.AluOpType.add,
                            )
                            nc.scalar.activation(
                                out=o_sb[:sw, pr, :], in_=t1[:sw, :], func=AF.Sigmoid
                            )
                        # one output DMA per (chunk, s-block): bulk on SWDGE,
                        # final chunk on the scalar ring for a short tail
                        oeng = nc.scalar if last_chunk else nc.gpsimd
                        oeng.dma_start(
                            out=ov[s0 : s0 + sw, ch * PRS : (ch + 1) * PRS, :],
                            in_=o_sb[:sw],
                        )

    nc.compile()
    return nc


def _get_nc(Cpt, Cpi):
    key = (Cpt, Cpi)
    if key not in _nc_cache:
        _nc_cache[key] = _build(Cpt, Cpi)
    return _nc_cache[key]


def _pack_weights(W, nexp):
    """[nexp, G, HID] f32 -> [nexp/_CPE, 128, _CPE*4*G] partition-major fp8
    chunks, scaled by _WSCALE and clipped to the e3m4 range."""
    nch = nexp // _CPE
    A = W.reshape(nch, _CPE, _G, 4, 128)          # [ch, e, g, c, p]
    A = np.ascontiguousarray(A.transpose(0, 4, 1, 3, 2))  # [ch, p, e, c, g]
    A = np.clip(A * _WSCALE, -15.5, 15.5)
    return A.reshape(nch, 128, _CPE * 4 * _G).astype(ml_dtypes.float8_e3m4)


def _route(ids, mask, n_experts):
    """Per-PAIR sample lists: pair j owns experts 2j, 2j+1. Returns
    (pair_samples, pair_parity) lists of arrays."""
    samples, parity = [], []
    for j in range(n_experts // 2):
        s0 = np.flatnonzero((ids == 2 * j) & mask)
        s1 = np.flatnonzero((ids == 2 * j + 1) & mask)
        samples.append(np.concatenate([s0, s1]))
        parity.append(np.concatenate([np.zeros(len(s0), np.int64),
                                      np.ones(len(s1), np.int64)]))
    return samples, parity


def kernel(
    feature,
    table_ids,
    index_ids,
    table_mask,
    index_mask,
    W_up,
    b_up,
    W_table,
    b_table,
    W_index,
    b_index,
):
    global LAST_RESULTS
    from concourse.bass_utils import run_bass_kernel_spmd

    feature = np.ascontiguousarray(np.asarray(feature), dtype=np.float32)
    table_ids = np.asarray(table_ids).astype(np.int64)
    index_ids = np.asarray(index_ids).astype(np.int64)
    table_mask = np.asarray(table_mask).astype(bool)
    index_mask = np.asarray(index_mask).astype(bool)
    W_up = np.asarray(W_up, dtype=np.float32)
    b_up = np.asarray(b_up, dtype=np.float32)
    W_table = np.asarray(W_table, dtype=np.float32)
    b_table = np.asarray(b_table, dtype=np.float32)
    W_index = np.asarray(W_index, dtype=np.float32)
    b_index = np.asarray(b_index, dtype=np.float32)

    B = feature.shape[0]

    smp_t, par_t = _route(table_ids, table_mask, _N_TABLES)
    smp_i, par_i = _route(index_ids, index_mask, _N_INDEXES)
    # Uniform per-pair capacity so all 8 cores run one identical program.
    Cpt = max(8, -(-max(len(s) for s in smp_t) // 8) * 8)
    Cpi = max(8, -(-max(len(s) for s in smp_i) // 8) * 8)

    nc = _get_nc(Cpt, Cpi)

    TP = _TPC // 2
    IP = _IPC // 2
    W_upT = np.ascontiguousarray(W_up.T / _WSCALE).astype(ml_dtypes.bfloat16)
    buc = np.ascontiguousarray((b_up / _WSCALE).reshape(4, 128).T)

    in_maps = []
    for c in range(_N_CORES):
        ts = slice(c * _TPC, (c + 1) * _TPC)
        is_ = slice(c * _IPC, (c + 1) * _IPC)
        fa_c = np.zeros((_F, TP * Cpt + IP * Cpi), ml_dtypes.bfloat16)
        ft_c = fa_c[:, : TP * Cpt]
        for j in range(TP):
            s = smp_t[c * TP + j]
            if len(s):
                ft_c[:, j * Cpt : j * Cpt + len(s)] = feature[s].T
        fi_c = fa_c[:, TP * Cpt :]
        for j in range(IP):
            s = smp_i[c * IP + j]
            if len(s):
                fi_c[:, j * Cpi : j * Cpi + len(s)] = feature[s].T
        bb_row = np.concatenate(
            [b_table[ts].reshape(-1), b_index[is_].reshape(-1)]
        ).astype(ml_dtypes.float8_e3m4)
        bb_c = np.ascontiguousarray(np.broadcast_to(bb_row, (128, bb_row.size)))
        in_maps.append(
            {
                "fa": fa_c,
                "wt": _pack_weights(W_table[ts], _TPC),
                "wi": _pack_weights(W_index[is_], _IPC),
                "bb": bb_c,
                "wu": W_upT,
                "buc": buc,
            }
        )

    res = run_bass_kernel_spmd(
        nc, in_maps, core_ids=list(range(_N_CORES)), trace=TRACE
    )
    LAST_RESULTS = res

    out = np.zeros((2, B, _G), np.float32)
    for c in range(_N_CORES):
        rt = res.results[c]["ot"].astype(np.float32)
        ri = res.results[c]["oi"].astype(np.float32)
        for j in range(TP):
            s = smp_t[c * TP + j]
            if len(s):
                rows = rt[j * Cpt : j * Cpt + len(s)].reshape(len(s), 2, _G)
                out[0, s, :] = rows[np.arange(len(s)), par_t[c * TP + j], :]
        for j in range(IP):
            s = smp_i[c * IP + j]
            if len(s):
                rows = ri[j * Cpi : j * Cpi + len(s)].reshape(len(s), 2, _G)
                out[1, s, :] = rows[np.arange(len(s)), par_i[c * IP + j], :]
    return out


# revision 10
# speedup vs baseline: 1.3490x; 1.2120x over previous
"""Trainium2 Bass kernel for nn_CachePredictor (moe_routing).

Computation (see reference):
    x = relu(feature @ W_up.T + b_up)                      [B, 512]
    t_out = sigmoid(einsum('bf,bgf', x, W_table[tids]) + b_table[tids]) * tmask
    i_out = sigmoid(einsum('bf,bgf', x, W_index[iids]) + b_index[iids]) * imask
    out = stack([t_out, i_out])                            [2, B, 256]

Strategy: expert sharding. Per-sample gather of expert weights would move
~4 GB of HBM traffic; grouping samples by expert reads each expert matrix
exactly once. Each of the 8 cores owns 8 table experts and 16 index
experts and processes only the samples routed to its experts. The host
computes routing metadata (sample->expert grouping, capacity padding) and
arranges per-core inputs; all FLOPs run on device.

v2 changes vs the 41 us baseline (toward the ~13 us DMA / ~14 us PE
rooflines):
- Expert weights stored in HBM as fp8 e3m4 (scaled x32, clipped to +-15.5)
  and fed STRAIGHT to the PE as the moving operand while x stays bf16
  (mixed-dtype matmul verified on HW). Halves the dominant HBM stream
  (6.3 -> 3.15 MB/core). The x32 is compensated exactly by folding /32
  into W_up/b_up (pure exponent shift in bf16).
- No warmup matmuls, no K=1 bias matmuls (~9 us of PE instructions in the
  baseline). Expert biases arrive host-pre-broadcast as a [128, 6144]
  fp8e3 input (0.77 MB, abs err <= ~5e-4 on biases ~1e-2); per pair the
  bias-add runs on the DVE (PSUM + bias -> SBUF f32) and the sigmoid on
  ACT (-> bf16). (A device-side gpsimd partition_broadcast was tried and
  measured 9.2 us + bad scheduling - it serialized all of stage 2.)
- Stage-1 relu+bias is split: m-chunks 0,1 on ACT (native Relu with
  per-partition bias), 2,3 on DVE - stage 1 was DVE-paced otherwise.
  Dummy 1-element Relu/Sigmoid ops pull the ~1.3 us ACT table loads off
  the critical path.
- wu rides the scalar ring while fa0 rides sync, so the first stage-1
  matmul's deps land in parallel right after the ~7 us engine preamble.
- Outputs written bf16 (host upcasts): halves output traffic.
- Weight chunks of 8 experts (1 MB fp8) stream on the sync HWDGE ring
  only, so ACT compute never delays a weight-DMA dispatch; bulk outputs
  ride SWDGE, the final chunk's output uses the scalar ring for a short
  tail.
- A dummy 1-element sigmoid early on ACT pulls the ~1.3 us activation
  table load into the startup window.

Per-PAIR processing (pair = 2 consecutive experts sharing one padded
column segment = union of both experts' samples): every sample is
multiplied against BOTH experts' weights in one N=512 moving pass and the
host keeps the valid half. Redundant FLOPs are cheap; PE instruction
count is what matters.

Masked-off samples are never routed (reference zeroes them); the host
scatters computed rows back and leaves the rest zero.
"""

import ml_dtypes
import numpy as np

_N_CORES = 8
_F = 256        # feature dim
_HID = 512      # up-projection width
_G = 256        # buckets
_N_TABLES = 64
_N_INDEXES = 128
_TPC = _N_TABLES // _N_CORES    # table experts per core (8)
_IPC = _N_INDEXES // _N_CORES   # index experts per core (16)
_CPE = 8                        # experts per weight chunk (1 MiB fp8)
_WSCALE = 32.0                  # fp8 weight scale (folded into W_up/b_up)

_nc_cache = {}

# Set by a test harness to capture HW profiles; harmless when unused.
TRACE = False
LAST_RESULTS = None


def _build(Cpt, Cpi):
    """Build + compile the SPMD program for per-PAIR capacities (Cpt, Cpi)."""
    from concourse import bacc
    import concourse.tile as tile
    import concourse.mybir as mybir

    F32 = mybir.dt.float32
    BF16 = mybir.dt.bfloat16
    F8E3 = mybir.dt.float8e3
    AF = mybir.ActivationFunctionType

    TP = _TPC // 2   # table pairs per core (4)
    IP = _IPC // 2   # index pairs per core (8)
    NTcols = TP * Cpt
    NIcols = IP * Cpi
    TCH = _TPC // _CPE   # table weight chunks (1)
    ICH = _IPC // _CPE   # index weight chunks (2)
    PRS = _CPE // 2      # pairs per chunk (4)
    NB = (_TPC + _IPC) * _G   # bias columns (6144)

    nc = bacc.Bacc(
        "TRN2",
        target_bir_lowering=False,
        debug=False,
        enable_asserts=False,
        num_devices=_N_CORES,
    )
    fa = nc.dram_tensor("fa", [_F, NTcols + NIcols], BF16, kind="ExternalInput").ap()
    # host-packed, partition-major: [chunk, p, e_local*1024 + c*256 + g]
    wt = nc.dram_tensor("wt", [TCH, 128, _CPE * 4 * _G], F8E3, kind="ExternalInput").ap()
    wi = nc.dram_tensor("wi", [ICH, 128, _CPE * 4 * _G], F8E3, kind="ExternalInput").ap()
    # biases, host-pre-broadcast to all partitions: table pairs then index
    # pairs, [128, 6144] fp8e3
    bb = nc.dram_tensor("bb", [128, NB], F8E3, kind="ExternalInput").ap()
    wu = nc.dram_tensor("wu", [_F, _HID], BF16, kind="ExternalInput").ap()  # W_up.T/32
    buc = nc.dram_tensor("buc", [128, 4], F32, kind="ExternalInput").ap()  # b_up/32 col-major
    # outputs: per pair, both experts' logits for every sample in the segment
    ot = nc.dram_tensor("ot", [NTcols, 2 * _G], BF16, kind="ExternalOutput").ap()
    oi = nc.dram_tensor("oi", [NIcols, 2 * _G], BF16, kind="ExternalOutput").ap()

    otv = ot.rearrange("(j s) g -> s j g", s=Cpt)
    oiv = oi.rearrange("(j s) g -> s j g", s=Cpi)

    with tile.TileContext(nc) as tc:
        with (
            tc.tile_pool(name="persist", bufs=1) as persist,
            tc.tile_pool(name="wpool", bufs=3) as wpool,
            tc.tile_pool(name="t1pool", bufs=4) as t1pool,
            tc.tile_pool(name="opool", bufs=3) as opool,
            tc.tile_pool(name="ps1pool", bufs=4, space="PSUM") as ps1pool,
            tc.tile_pool(name="ps2pool", bufs=4, space="PSUM") as ps2pool,
        ):
            # The SDMA queue arbitration behaves strict-priority: the sync
            # ring starves the scalar ring while it has work. So the WHOLE
            # latency chain (wu/fa/buc, then weight chunks in consumption
            # order) rides the sync ring alone; bbc goes out first on SWDGE
            # (its data moves before the weight stream starts); the scalar
            # ring carries only the final outputs (sync ring idle by then).
            NA = NTcols + NIcols
            bbc = persist.tile([128, NB], F8E3, name="bbc", tag="bbc")
            nc.gpsimd.dma_start(out=bbc, in_=bb)
            wu_sb = persist.tile([128, 2, _HID], BF16, name="wu_sb", tag="wu_sb")
            nc.sync.dma_start(out=wu_sb, in_=wu.rearrange("(c p) m -> p c m", p=128))
            f_sb = []
            for c in range(2):
                f_c = persist.tile([128, NA], BF16, name=f"f_a{c}", tag=f"f_a{c}")
                nc.sync.dma_start(out=f_c, in_=fa[c * 128 : (c + 1) * 128, :])
                f_sb.append(f_c)
            buc_sb = persist.tile([128, 4], F32, name="buc_sb", tag="buc_sb")
            nc.sync.dma_start(out=buc_sb, in_=buc)

            # pull the ACT relu table load into the startup window
            dummy = persist.tile([1, 16], F32, name="dummy", tag="dummy")
            nc.vector.memset(dummy, 0.0)
            nc.scalar.activation(out=dummy, in_=dummy, func=AF.Relu)

            # PE warmup during the fa-DMA wait: ~4.5 us of dummy matmuls
            # ramp the HAM clock gate to 2.4 GHz before stage 1 arrives
            warm = persist.tile([128, 512], BF16, name="warm", tag="warm")
            nc.vector.memset(warm, 0.0)
            for _ in range(10):
                psw = ps1pool.tile([128, 512], F32, name="ps1", tag="ps1")
                nc.tensor.matmul(psw, lhsT=warm[:, :128], rhs=warm, start=True, stop=True)

            # stage 1: xT[512, cols] = relu(W_upT.T @ featT + b_up), bf16,
            # paced by DVE relu+bias (one fused tensor_scalar per m-chunk)
            x_sb = {}
            off = {"t": 0, "i": NTcols}
            for role, NC in (("t", NTcols), ("i", NIcols)):
                x_sb[role] = [
                    persist.tile(
                        [128, NC], BF16, name=f"x_{role}{m}", tag=f"x_{role}{m}"
                    )
                    for m in range(4)
                ]
            # interleave m-chunk pairs so consecutive matmuls hit different
            # PSUM banks (same-bank accumulation passes serialize the PE)
            for role, NC in (("t", NTcols), ("i", NIcols)):
                for n0 in range(0, NC, 512):
                    nw = min(512, NC - n0)
                    for m0 in (0, 2):
                        ps1s = {
                            m: ps1pool.tile([128, 512], F32, name="ps1", tag="ps1")
                            for m in (m0, m0 + 1)
                        }
                        for c in range(2):
                            for m in (m0, m0 + 1):
                                nc.tensor.matmul(
                                    ps1s[m][:, :nw],
                                    lhsT=wu_sb[:, c, m * 128 : (m + 1) * 128],
                                    rhs=f_sb[c][:, off[role] + n0 : off[role] + n0 + nw],
                                    start=(c == 0),
                                    stop=(c == 1),
                                )
                        for m in (m0, m0 + 1):
                            # relu+bias split across ACT (native Relu with
                            # per-partition bias) and DVE so neither engine
                            # paces stage 1 alone
                            if m < 2:
                                nc.scalar.activation(
                                    out=x_sb[role][m][:, n0 : n0 + nw],
                                    in_=ps1s[m][:, :nw],
                                    func=AF.Relu,
                                    bias=buc_sb[:, m : m + 1],
                                )
                            else:
                                nc.vector.tensor_scalar(
                                    out=x_sb[role][m][:, n0 : n0 + nw],
                                    in0=ps1s[m][:, :nw],
                                    scalar1=buc_sb[:, m : m + 1],
                                    scalar2=0.0,
                                    op0=mybir.AluOpType.add,
                                    op1=mybir.AluOpType.max,
                                )

            # preload the sigmoid table while stage-2 matmuls run
            nc.scalar.activation(out=dummy, in_=dummy, func=AF.Sigmoid)

            # stage 2: weight chunks of 8 experts = 4 pairs. One pair => one
            # column segment, one PSUM bank, 4 K-chunk matmuls with rhs
            # spanning both experts (N=512, fp8), DVE bias add, ACT sigmoid.
            for role, wdram, ov, nch, C, boff in (
                ("t", wt, otv, TCH, Cpt, 0),
                ("i", wi, oiv, ICH, Cpi, _TPC * _G),
            ):
                xs = x_sb[role]
                for ch in range(nch):
                    w_sb = wpool.tile(
                        [128, _CPE, 4, _G], F8E3, name=f"w_sb_{role}", tag="w_sb"
                    )
                    wv = wdram[ch].rearrange("p (e c g) -> p e c g", e=_CPE, c=4)
                    # whole chunk on the sync ring: ACT compute never delays
                    # weight streaming
                    nc.sync.dma_start(out=w_sb, in_=wv)
                    last_chunk = role == "i" and ch == nch - 1
                    for s0 in range(0, C, 128):
                        sw = min(128, C - s0)
                        o_sb = opool.tile(
                            [128, PRS, 2 * _G], BF16, name="o_sb", tag="o_sb"
                        )
                        for pr in range(PRS):
                            j = ch * PRS + pr
                            ps2 = ps2pool.tile([128, 512], F32, name="ps2", tag="ps2")
                            for c in range(4):
                                nc.tensor.matmul(
                                    ps2[:sw, :],
                                    lhsT=xs[c][:, j * C + s0 : j * C + s0 + sw],
                                    rhs=w_sb[:, 2 * pr : 2 * pr + 2, c, :],
                                    start=(c == 0),
                                    stop=(c == 3),
                                )
                            t1 = t1pool.tile([128, 512], F32, name="t1", tag="t1")
                            nc.vector.tensor_tensor(
                                out=t1[:sw, :],
                                in0=ps2[:sw, :],
                                in1=bbc[:sw, boff + j * 512 : boff + (j + 1) * 512],
                                op=mybir.AluOpType.add,
                            )
                            nc.scalar.activation(
                                out=o_sb[:sw, pr, :], in_=t1[:sw, :], func=AF.Sigmoid
                            )
                            if last_chunk:
                                # final chunk: per-pair DMAs on the (idle)
                                # scalar ring so the tail is one small DMA
                                # after the last sigmoid, not a 4-pair batch
                                nc.scalar.dma_start(
                                    out=ov[s0 : s0 + sw, j : j + 1, :],
                                    in_=o_sb[:sw, pr : pr + 1, :],
                                )
                        if not last_chunk:
                            # bulk outputs ride SWDGE, one DMA per (chunk,
                            # s-block)
                            nc.gpsimd.dma_start(
                                out=ov[s0 : s0 + sw, ch * PRS : (ch + 1) * PRS, :],
                                in_=o_sb[:sw],
                            )

    nc.compile()
    return nc


def _get_nc(Cpt, Cpi):
    key = (Cpt, Cpi)
    if key not in _nc_cache:
        _nc_cache[key] = _build(Cpt, Cpi)
    return _nc_cache[key]


def _pack_weights(W, nexp):
    """[nexp, G, HID] f32 -> [nexp/_CPE, 128, _CPE*4*G] partition-major fp8
    chunks, scaled by _WSCALE and clipped to the e3m4 range."""
    nch = nexp // _CPE
    A = W.reshape(nch, _CPE, _G, 4, 128)          # [ch, e, g, c, p]
    A = np.ascontiguousarray(A.transpose(0, 4, 1, 3, 2))  # [ch, p, e, c, g]
    A = np.clip(A * _WSCALE, -15.5, 15.5)
    return A.reshape(nch, 128, _CPE * 4 * _G).astype(ml_dtypes.float8_e3m4)


def _route(ids, mask, n_experts):
    """Per-PAIR sample lists: pair j owns experts 2j, 2j+1. Returns
    (pair_samples, pair_parity) lists of arrays."""
    samples, parity = [], []
    for j in range(n_experts // 2):
        s0 = np.flatnonzero((ids == 2 * j) & mask)
        s1 = np.flatnonzero((ids == 2 * j + 1) & mask)
        samples.append(np.concatenate([s0, s1]))
        parity.append(np.concatenate([np.zeros(len(s0), np.int64),
                                      np.ones(len(s1), np.int64)]))
    return samples, parity


def kernel(
    feature,
    table_ids,
    index_ids,
    table_mask,
    index_mask,
    W_up,
    b_up,
    W_table,
    b_table,
    W_index,
    b_index,
):
    global LAST_RESULTS
    from concourse.bass_utils import run_bass_kernel_spmd

    feature = np.ascontiguousarray(np.asarray(feature), dtype=np.float32)
    table_ids = np.asarray(table_ids).astype(np.int64)
    index_ids = np.asarray(index_ids).astype(np.int64)
    table_mask = np.asarray(table_mask).astype(bool)
    index_mask = np.asarray(index_mask).astype(bool)
    W_up = np.asarray(W_up, dtype=np.float32)
    b_up = np.asarray(b_up, dtype=np.float32)
    W_table = np.asarray(W_table, dtype=np.float32)
    b_table = np.asarray(b_table, dtype=np.float32)
    W_index = np.asarray(W_index, dtype=np.float32)
    b_index = np.asarray(b_index, dtype=np.float32)

    B = feature.shape[0]

    smp_t, par_t = _route(table_ids, table_mask, _N_TABLES)
    smp_i, par_i = _route(index_ids, index_mask, _N_INDEXES)
    # Uniform per-pair capacity so all 8 cores run one identical program.
    Cpt = max(8, -(-max(len(s) for s in smp_t) // 8) * 8)
    Cpi = max(8, -(-max(len(s) for s in smp_i) // 8) * 8)

    nc = _get_nc(Cpt, Cpi)

    TP = _TPC // 2
    IP = _IPC // 2
    W_upT = np.ascontiguousarray(W_up.T / _WSCALE).astype(ml_dtypes.bfloat16)
    buc = np.ascontiguousarray((b_up / _WSCALE).reshape(4, 128).T)

    in_maps = []
    for c in range(_N_CORES):
        ts = slice(c * _TPC, (c + 1) * _TPC)
        is_ = slice(c * _IPC, (c + 1) * _IPC)
        fa_c = np.zeros((_F, TP * Cpt + IP * Cpi), ml_dtypes.bfloat16)
        ft_c = fa_c[:, : TP * Cpt]
        for j in range(TP):
            s = smp_t[c * TP + j]
            if len(s):
                ft_c[:, j * Cpt : j * Cpt + len(s)] = feature[s].T
        fi_c = fa_c[:, TP * Cpt :]
        for j in range(IP):
            s = smp_i[c * IP + j]
            if len(s):
                fi_c[:, j * Cpi : j * Cpi + len(s)] = feature[s].T
        bb_row = np.concatenate(
            [b_table[ts].reshape(-1), b_index[is_].reshape(-1)]
        ).astype(ml_dtypes.float8_e3m4)
        bb_c = np.ascontiguousarray(np.broadcast_to(bb_row, (128, bb_row.size)))
        in_maps.append(
            {
                "fa": fa_c,
                "wt": _pack_weights(W_table[ts], _TPC),
                "wi": _pack_weights(W_index[is_], _IPC),
                "bb": bb_c,
                "wu": W_upT,
                "buc": buc,
            }
        )

    res = run_bass_kernel_spmd(
        nc, in_maps, core_ids=list(range(_N_CORES)), trace=TRACE
    )
    LAST_RESULTS = res

    out = np.zeros((2, B, _G), np.float32)
    for c in range(_N_CORES):
        rt = res.results[c]["ot"].astype(np.float32)
        ri = res.results[c]["oi"].astype(np.float32)
        for j in range(TP):
            s = smp_t[c * TP + j]
            if len(s):
                rows = rt[j * Cpt : j * Cpt + len(s)].reshape(len(s), 2, _G)
                out[0, s, :] = rows[np.arange(len(s)), par_t[c * TP + j], :]
        for j in range(IP):
            s = smp_i[c * IP + j]
            if len(s):
                rows = ri[j * Cpi : j * Cpi + len(s)].reshape(len(s), 2, _G)
                out[1, s, :] = rows[np.arange(len(s)), par_i[c * IP + j], :]
    return out
